# revision 26
# baseline (speedup 1.0000x reference)
"""Trainium2 Bass kernel for nn_DQNConv (conv stack -> linear -> legal-move
masked softmax), data-parallel over 8 NeuronCores.

Self-contained: takes FULL inputs as numpy arrays, shards batch across the 8
cores, runs one SPMD Bass program, returns the FULL [16384, 4096] float32
output.

Device computes, per core (2048 rows): the three VALID 3x3 convs as dense
fp16 matmuls (features on the SBUF partition dim, batch on the free dim:
7x7x1 -> 800 -> 576 -> 64), then the dense logits [128 rows, 4096] =
feat.T @ Wl.T, shipped to HBM as RAW fp16 logits. Contraction sizes are
tuned to the measured TRN2 PE timing (N=512-col matmuls: K=96 -> 225ns,
K=112 -> 233ns, K=128&M=128 -> 268ns, K<=65 -> ~2x penalty):

  * x/M1 are zero-padded to K=96 (L1: 7 matmuls K=96).
  * h1's 800 features are tiled {112x6, 128} so L2's k-tiles run at K=112.
  * h2 is padded to 640 = 5x128; L3 runs K=128/M=96, and the M=96 feat
    output leaves rows 64:96 zero -- exactly the padding the K=96 logits
    matmuls need (lhsT = feat[:, 128r:128r+128], rhs = Wl.T padded to 96).

PSUM->SBUF evacuation is plain dtype-converting copies (relu via max for
the conv layers) split across DVE and ACT so neither engine gates the PE;
logits quarters of chunk c-1 are interleaved into chunk c's conv matmul
stream (in-order PE never parks on a PSUM-slot wait).

The masked softmax is pure index marshalling on ~64 values/row: the host
gathers the 64 legal fp16 logits per row, exponentiates in f32 (fp16
logit quantization is 2^-11 relative, ~10x tighter than the 2e-2 gate
needs), sums the distinct ones (duplicate occurrences of the same move
contribute once, matching the reference's scatter), normalizes, and
scatters into the zero-initialized dense output.
"""

import sys
import os

for _p in ("/opt/trn_rl_repo", "/root/.axon_site/_ro/trn_rl_repo"):
    if os.path.isdir(_p) and _p not in sys.path:
        sys.path.append(_p)

import numpy as np

import concourse.bass as bass
import concourse.bacc as bacc
import concourse.mybir as mybir
import concourse.tile as tile
from concourse.bass_utils import run_bass_kernel_spmd

B, HW, OUT, K = 16384, 7, 4096, 64
NCORES = 8
BC = B // NCORES           # 2048 rows per core
NRT = BC // 128            # 16 row-tiles per core
NCHUNK = 4                 # conv batch chunks per core
CW = BC // NCHUNK          # 512 columns per conv chunk
F0, F1, F2, F3 = 49, 800, 576, 64
KP = 96                    # padded contraction for L1 / logits
F2P = 640                  # h2 padded to 5x128
T1 = [(0, 112), (112, 112), (224, 112), (336, 112),
      (448, 112), (560, 112), (672, 128)]          # h1 k/m tiles
T2 = [(0, 128), (128, 128), (256, 128), (384, 128), (512, 128)]  # h2 tiles

dt = mybir.dt
AT = mybir.AluOpType
ACTF = mybir.ActivationFunctionType
FP16 = dt.float16


def _build(reps=1, fori=0, phase="full"):
    nc = bacc.Bacc("TRN2", target_bir_lowering=False, debug=False)

    xT = nc.dram_tensor("xT", [KP, BC], FP16, kind="ExternalInput")
    m1 = nc.dram_tensor("m1", [KP, F1], FP16, kind="ExternalInput")
    # m2/m3 arrive pre-packed into partition-tile blocks (one DMA each)
    m2 = nc.dram_tensor("m2", [128, 7 * F2P], FP16, kind="ExternalInput")
    m3 = nc.dram_tensor("m3", [128, 5 * KP], FP16, kind="ExternalInput")
    wlT = nc.dram_tensor("wlT", [KP, OUT], FP16, kind="ExternalInput")
    outd = nc.dram_tensor("out", [BC, OUT], FP16, kind="ExternalOutput")

    cp_bufs, lp_bufs = 4, 2
    op_bufs = 16 if phase == "fullop16" else 8
    with tile.TileContext(nc) as tc:
        with (
            tc.tile_pool(name="w", bufs=1) as wp,
            tc.tile_pool(name="h", bufs=2) as hp,
            tc.tile_pool(name="o", bufs=op_bufs) as op,
            tc.tile_pool(name="cp", bufs=cp_bufs, space="PSUM") as cp,
            tc.tile_pool(name="lp", bufs=lp_bufs, space="PSUM") as lp,
        ):
            # ---- static loads -------------------------------------------------
            # load order: L1's operands (m1 + x chunk 0) first so compute
            # starts ~1us in; the bulk of x lands while chunk 0 runs
            m1_sb = wp.tile([KP, F1], FP16, tag="m1")
            nc.sync.dma_start(out=m1_sb[:], in_=m1.ap())
            xT_sb = wp.tile([KP, BC], FP16, tag="xT")
            nc.sync.dma_start(out=xT_sb[:, 0:CW], in_=xT.ap()[:, 0:CW])
            m2_all = wp.tile([128, 7 * F2P], FP16, tag="m2a")
            nc.sync.dma_start(out=m2_all[:], in_=m2.ap())
            # block i: [kn_i, 640] = M2 rows kb..kb+kn (cols 576:640 zero)
            m2_sb = [m2_all[:kn, i * F2P:(i + 1) * F2P]
                     for i, (kb, kn) in enumerate(T1)]
            m3_all = wp.tile([128, 5 * KP], FP16, tag="m3a")
            nc.sync.dma_start(out=m3_all[:], in_=m3.ap())
            # block k: [128, 96] = M3 rows of h2 tile k (cols 64:96 zero)
            m3_sb = [m3_all[:, k * KP:(k + 1) * KP] for k in range(5)]
            wl_sb = wp.tile([KP, OUT], FP16, tag="wl")
            nc.sync.dma_start(out=wl_sb[:], in_=wlT.ap())
            nc.sync.dma_start(out=xT_sb[:, CW:], in_=xT.ap()[:, CW:])
            dma_src = []
            if phase in ("dmaonly", "dmafree"):
                for i in range(4):
                    t = wp.tile([128, OUT], FP16, tag=f"dsrc{i}")
                    nc.vector.memset(t[:], 0.25)
                    dma_src.append(t)

            # ---- per-chunk conv + per-row-tile logits -------------------------
            pending = []       # quarter-emitters from the previous chunk
            tick_n = [0]

            def emit_quarter(endgame=False):
                rt, q, lhsT, o = pending.pop(0)
                psl = lp.tile([128, 1024], dt.float32, tag="psl")
                for nb in range(2):
                    nc.tensor.matmul(
                        psl[:, nb * 512:(nb + 1) * 512],
                        lhsT,
                        wl_sb[:, q * 1024 + nb * 512:q * 1024 + (nb + 1) * 512],
                        start=True, stop=True,
                    )
                osl = o[:, q * 1024:(q + 1) * 1024]
                # raw-logit copy evacuation, split DVE (q0) / ACT (q1-3);
                # endgame (no conv matmuls left to interleave): alternate
                # DVE/ACT so neither engine serializes the drain
                if endgame:
                    use_dve = q in (0, 2)
                elif phase == "full22":
                    use_dve = q in (0, 1)
                else:
                    use_dve = q == 0
                if use_dve:
                    nc.vector.tensor_scalar(
                        out=osl, in0=psl[:],
                        scalar1=1.0, scalar2=None, op0=AT.mult)
                else:
                    nc.scalar.activation(osl, psl[:], ACTF.Copy)
                if phase == "fullq3":
                    if q == 3:
                        nc.sync.dma_start(
                            out=outd.ap()[rt * 128:(rt + 1) * 128, :],
                            in_=o[:])
                elif q in (1, 3) and phase != "noout":
                    src = dma_src[rt % 4] if phase == "dmafree" else o
                    hb = (q - 1) * 1024
                    nc.sync.dma_start(
                        out=outd.ap()[rt * 128:(rt + 1) * 128, hb:hb + 2048],
                        in_=src[:, hb:hb + 2048])

            def tick(stride=3):
                tick_n[0] += 1
                if pending and tick_n[0] % stride == 0:
                    emit_quarter()

            # fori>0 wraps the body in a hardware loop (timing-only path)
            import contextlib
            _loop = tc.For_i(0, fori, 1) if fori > 0 else contextlib.nullcontext()
            with _loop:
             for _rep in range(reps):
              if phase == "dmaonly":
                for rt in range(NRT):
                    nc.sync.dma_start(
                        out=outd.ap()[rt * 128:(rt + 1) * 128, :],
                        in_=dma_src[rt % 4][:])
                continue
              for c in range(NCHUNK):
                cs = slice(c * CW, (c + 1) * CW)

                # L1: h1 = relu(x@M1), tiled {112x6, 128}, K=96
                h1 = []
                for i, (kb, kn) in enumerate(T1):
                    ps = cp.tile([kn, CW], dt.float32, tag="cps",
                                 name=f"ps1_{i}")
                    nc.tensor.matmul(
                        ps[:],
                        m1_sb[:, kb:kb + kn],
                        xT_sb[:, cs],
                        start=True, stop=True,
                    )
                    tick(stride=5 if phase == "fulladpt" else 3)
                    h = hp.tile([kn, CW], FP16, tag=f"h1_{i}")
                    nc.vector.tensor_scalar(
                        out=h[:], in0=ps[:],
                        scalar1=0.0, scalar2=None, op0=AT.max)
                    h1.append(h)

                # L2: h2 = relu(h1@M2), 5 m-tiles of 128, k-tiles = T1
                h2 = []
                for m in range(5):
                    ps = cp.tile([128, CW], dt.float32, tag="cps",
                                 name=f"ps2_{m}")
                    for kt in range(7):
                        nc.tensor.matmul(
                            ps[:],
                            m2_sb[kt][:, m * 128:(m + 1) * 128],
                            h1[kt][:],
                            start=(kt == 0), stop=(kt == 6),
                        )
                        tick(stride=2 if phase == "fulladpt" else 3)
                    h = hp.tile([128, CW], FP16, tag=f"h2_{m}")
                    nc.vector.tensor_scalar(
                        out=h[:], in0=ps[:],
                        scalar1=0.0, scalar2=None, op0=AT.max)
                    h2.append(h)

                # L3: feat[96, CW] = relu(h2@M3); rows 64:96 stay zero and
                # provide the K=96 padding for the logits matmuls.
                ps3 = cp.tile([KP, CW], dt.float32, tag="cps")
                for kt in range(5):
                    nc.tensor.matmul(
                        ps3[:],
                        m3_sb[kt],
                        h2[kt][:],
                        start=(kt == 0), stop=(kt == 4),
                    )
                    tick()
                feat = hp.tile([KP, CW], FP16, tag="feat")
                nc.vector.tensor_scalar(
                    out=feat[:], in0=ps3[:],
                    scalar1=0.0, scalar2=None, op0=AT.max)

                # queue this chunk's logits quarters (emitted via tick()
                # interleaved into the NEXT chunk's conv stream)
                while pending:         # leftovers from chunk c-1
                    emit_quarter()
                for r in range(CW // 128):
                    rt = c * (CW // 128) + r
                    lhsT = feat[:, r * 128:(r + 1) * 128]
                    if phase in ("conv", "convdeep"):
                        od = op.tile([F3, CW], FP16, tag="o",
                                     name=f"od_{rt}")
                        nc.vector.tensor_scalar(
                            out=od[:, :CW], in0=feat[:F3, :],
                            scalar1=1.0, scalar2=None, op0=AT.mult)
                        nc.sync.dma_start(
                            out=outd.ap()[rt * 64:(rt + 1) * 64, :CW],
                            in_=od[:, :CW])
                        continue
                    o = op.tile([128, OUT], FP16, tag="o", name=f"o_{rt}")
                    for q in range(4):
                        pending.append((rt, q, lhsT, o))
              while pending:
                emit_quarter(endgame=True)

    nc.compile()
    return nc


_CACHE = {}


def _get_nc(reps=1, fori=0, phase="full"):
    key = ("nc", reps, fori, phase)
    if key not in _CACHE:
        _CACHE[key] = _build(reps, fori, phase)
    return _CACHE[key]


def _conv_mats(W1, W2, W3):
    """Dense [in_feat, out_feat] matrices for the three VALID 3x3 convs with
    channel-major (c, y, x) feature flattening on both sides."""
    M1 = np.zeros((F0, F1), np.float32)
    for ky in range(3):
        for kx in range(3):
            for oy in range(5):
                for ox in range(5):
                    # row = input pixel, col = (oc, oy, ox)
                    M1[(oy + ky) * 7 + (ox + kx),
                       np.arange(32) * 25 + oy * 5 + ox] = W1[:, 0, ky, kx]
    M2 = np.zeros((F1, F2), np.float32)
    ic = np.arange(32)
    for ky in range(3):
        for kx in range(3):
            for oy in range(3):
                for ox in range(3):
                    rows = ic * 25 + (oy + ky) * 5 + (ox + kx)      # [32]
                    cols = np.arange(64) * 9 + oy * 3 + ox           # [64]
                    M2[np.ix_(rows, cols)] = W2[:, :, ky, kx].T      # [32,64]
    M3 = W3.transpose(1, 2, 3, 0).reshape(F2, F3).astype(np.float32)
    return M1, M2, M3


def _prep_in_maps(inputs):
    x = np.ascontiguousarray(np.asarray(inputs["x"], dtype=np.float32)).reshape(B, F0)
    W1 = np.asarray(inputs["W1"], dtype=np.float32)
    W2 = np.asarray(inputs["W2"], dtype=np.float32)
    W3 = np.asarray(inputs["W3"], dtype=np.float32)
    Wl = np.asarray(inputs["Wl"], dtype=np.float32)

    M1, M2, M3 = _conv_mats(W1, W2, W3)

    M1p = np.zeros((KP, F1), np.float16)
    M1p[:F0] = M1
    M2p = np.zeros((128, 7 * F2P), np.float16)
    for i, (kb, kn) in enumerate(T1):
        M2p[:kn, i * F2P:i * F2P + F2] = M2[kb:kb + kn]
    M3p = np.zeros((128, 5 * KP), np.float16)
    for k in range(5):
        kn = min(128, F2 - 128 * k)
        M3p[:kn, k * KP:k * KP + F3] = M3[128 * k:128 * k + kn]
    WlTp = np.zeros((KP, OUT), np.float16)
    WlTp[:F3] = Wl.T
    xTp = np.zeros((KP, B), np.float16)
    xTp[:F0] = x.T

    in_maps = []
    for c in range(NCORES):
        sl = slice(c * BC, (c + 1) * BC)
        in_maps.append({
            "xT": np.ascontiguousarray(xTp[:, sl]),
            "m1": M1p, "m2": M2p, "m3": M3p, "wlT": WlTp,
        })
    return in_maps


def kernel(**inputs):
    pm = np.asarray(inputs["possible_moves"]).astype(np.int64, copy=False)
    in_maps = _prep_in_maps(inputs)

    nc = _get_nc()
    trace = bool(int(os.environ.get("KERNEL_TRACE", "0")))
    res = run_bass_kernel_spmd(nc, in_maps, list(range(NCORES)), trace=trace)
    _CACHE["last_results"] = res
    lg = np.concatenate([res.results[i]["out"] for i in range(NCORES)], axis=0)

    # ---- host-side masked softmax on the <=64 legal entries per row --------
    rows = np.arange(B)[:, None]
    e = np.exp(lg[rows, pm].astype(np.float32))      # [B, 64] legal exp(logit)
    # zero-weight duplicate occurrences so each distinct move counts once in Z
    srt = np.sort(pm, axis=1)
    order = np.argsort(pm, axis=1, kind="stable")
    dup_sorted = np.zeros(pm.shape, dtype=bool)
    dup_sorted[:, 1:] = srt[:, 1:] == srt[:, :-1]
    w = np.ones(pm.shape, np.float32)
    rr, _ = np.nonzero(dup_sorted)
    w[rr, order[dup_sorted]] = 0.0
    Z = np.einsum("ij,ij->i", e, w)
    vals = e / Z[:, None]
    out = np.zeros((B, OUT), np.float32)
    out[rows, pm] = vals       # duplicate indices write identical values
    return out
